# revision 6
# baseline (speedup 1.0000x reference)
"""GAT layer (masked additive-attention softmax + matmul + layernorm + leaky)
as a Trainium2 Bass kernel, data-parallel over batch across 8 NeuronCores.

Math: with s[i,j] = Wh1[i] + Wh2[j],
  z[i,j] = adj[i,j] * exp(leaky(s)) ;  out = LN((z @ Wh) / z.sum(-1)) -> leaky
Key factorization: exp(leaky(s)) = e^{0.2Wh1[i]} * e^{0.2Wh2[j]} * max(rho_i*rho_j, 1)
with rho = e^{0.8*Wh}, because e^{0.8s} >= 1 <=> s >= 0.  The row factor
e^{0.2Wh1[i]} cancels in the softmax ratio; the column factor e^{0.2Wh2[j]}
is folded into the matmul rhs R[j,:] = e^{0.2Wh2[j]} * [Wh[j,:] | 1].
So per (i,j) element we only compute G = min(max(rho_i*rho_j, 1), BIG*adj),
in fp16, with one tensor_scalar + one scalar_tensor_tensor DVE op per tile.
G^T tiles (for the PE matmul lhsT) come from PE transposes of DMA-cast
(int32 -> fp16) adjacency column-panels.  The layernorm uses
(y - mu)/sqrt(var + eps*D^2) == LN(y/D) to avoid normalizing by D first.
"""
import sys

sys.path.insert(0, "/opt/trn_rl_repo")

import numpy as np

import concourse.bass as bass
import concourse.mybir as mybir
import concourse.tile as tile
from concourse.bass_utils import run_bass_kernel_spmd
from concourse.masks import make_identity

dt = mybir.dt
A = mybir.ActivationFunctionType
Op = mybir.AluOpType

N = 2048          # nodes per batch
F = 128           # feature dim (in == out)
T = N // 128      # 16 tiles
EPS = 1e-5
ALPHA = 0.2
BIG = 30000.0     # > max possible mx value (~e^8), < fp16 max
N_CORES = 8


def _fix_sync_waits(nc, max_waits=1):
    """walrus in this container rejects >1 sync wait per instruction; spill
    excess waits onto same-engine no-ops inserted just before."""
    k = 0
    for f in nc.m.functions:
        for blk in f.blocks:
            insts = blk.instructions
            i = 0
            while i < len(insts):
                inst = insts[i]
                si = inst.sync_info
                if si is not None and len(si.on_wait) > max_waits:
                    waits = list(si.on_wait)
                    inst.sync_info = mybir.SyncInfo(
                        on_wait=waits[:max_waits], on_update=list(si.on_update))
                    pos = i
                    for w in waits[max_waits:]:
                        nop = mybir.InstNoOp(name=f"wait_spill_{k}", ins=[], outs=[])
                        k += 1
                        nop.engine = inst.engine
                        nop.sync_info = mybir.SyncInfo(on_wait=[w], on_update=[])
                        insts.insert(pos, nop)
                        pos += 1
                        i += 1
                i += 1


def build_gat_nc(fix_waits=True):
    nc = bass.Bass()
    H = nc.dram_tensor("h", [N, F], dt.float32, kind="ExternalInput")
    ADJ = nc.dram_tensor("adj", [N, N], dt.int32, kind="ExternalInput")
    WW = nc.dram_tensor("W_w", [F, F], dt.float32, kind="ExternalInput")
    WB = nc.dram_tensor("W_b", [F, 1], dt.float32, kind="ExternalInput")
    AW = nc.dram_tensor("a_w", [2, F], dt.float32, kind="ExternalInput")
    OUT = nc.dram_tensor("out", [N, F], dt.float32, kind="ExternalOutput")

    with tile.TileContext(nc) as tc:
      with tc.tile_pool(name="const", bufs=1) as const:
        with tc.tile_pool(name="spool", bufs=1) as spool, \
             tc.tile_pool(name="pp", bufs=2, space="PSUM") as pp:

            ident = const.tile([128, 128], dt.float32)
            make_identity(nc, ident[:])
            identh = const.tile([128, 128], dt.float16)
            make_identity(nc, identh[:])
            ones_row = const.tile([1, 128], dt.float32)
            nc.vector.memset(ones_row[:], 1.0)

            # ---- loads ----
            hs = spool.tile([128, T, F], dt.float32)      # hs[p,t,f] = h[t*128+p, f]
            nc.sync.dma_start(out=hs[:], in_=H[:].rearrange("(t p) f -> p t f", p=128))
            Ww_sb = const.tile([F, F], dt.float32)
            nc.sync.dma_start(out=Ww_sb[:], in_=WW[:])
            Wb_col = const.tile([F, 1], dt.float32)
            nc.sync.dma_start(out=Wb_col[:], in_=WB[:])
            aw_sb = const.tile([2, F], dt.float32)
            nc.sync.dma_start(out=aw_sb[:], in_=AW[:])

            # ---- hT tiles [f, n] and WwT [f, o] via PE transpose ----
            hT = spool.tile([128, N], dt.float32)
            for it in range(T):
                pt = pp.tile([128, 128], dt.float32, tag="ps", name="ps")
                nc.tensor.transpose(out=pt[:], in_=hs[:, it, :], identity=ident[:])
                nc.vector.tensor_copy(out=hT[:, it * 128:(it + 1) * 128], in_=pt[:])
            ptw = pp.tile([128, 128], dt.float32, tag="ps", name="ps")
            nc.tensor.transpose(out=ptw[:], in_=Ww_sb[:], identity=ident[:])
            WwT = const.tile([128, 128], dt.float32)
            nc.vector.tensor_copy(out=WwT[:], in_=ptw[:])

            # ---- WhT[o, n] = Ww @ h^T + b (bias via ACT Identity) ----
            WhT = spool.tile([128, N], dt.float32)
            for c in range(4):
                pw = pp.tile([128, 512], dt.float32, tag="ps", name="ps")
                nc.tensor.matmul(out=pw[:], lhsT=WwT[:], rhs=hT[:, c * 512:(c + 1) * 512],
                                 start=True, stop=True)
                nc.scalar.activation(out=WhT[:, c * 512:(c + 1) * 512], in_=pw[:],
                                     func=A.Identity, bias=Wb_col[:], scale=1.0)

            # ---- attention projections Wh1, Wh2 as rows then columns ----
            pa = pp.tile([128, 2], dt.float32, tag="ps", name="ps")
            nc.tensor.transpose(out=pa[:], in_=aw_sb[:], identity=ident[0:2, 0:2])
            acols = const.tile([128, 2], dt.float32)
            nc.vector.tensor_copy(out=acols[:], in_=pa[:])

            rows = [spool.tile([1, N], dt.float32, tag=f"row{r}", name=f"row{r}") for r in range(2)]
            for r in range(2):
                for c in range(4):
                    pr = pp.tile([1, 512], dt.float32, tag="ps", name="ps")
                    nc.tensor.matmul(out=pr[:], lhsT=acols[:, r:r + 1],
                                     rhs=WhT[:, c * 512:(c + 1) * 512],
                                     start=True, stop=True)
                    nc.scalar.activation(out=rows[r][0:1, c * 512:(c + 1) * 512],
                                         in_=pr[:], func=A.Identity)

            colsp = pp.tile([128, 32], dt.float32, tag="ps", name="ps")
            for t in range(T):
                nc.tensor.transpose(out=colsp[:, t:t + 1],
                                    in_=rows[0][0:1, t * 128:(t + 1) * 128],
                                    identity=ident[0:1, 0:1])
                nc.tensor.transpose(out=colsp[:, 16 + t:16 + t + 1],
                                    in_=rows[1][0:1, t * 128:(t + 1) * 128],
                                    identity=ident[0:1, 0:1])
            cols_sb = const.tile([128, 32], dt.float32)
            nc.vector.tensor_copy(out=cols_sb[:], in_=colsp[:])
            wh2cols = cols_sb[:, 16:32]

            # ---- per-j factors: rho_j, 1/rho_j, e^{0.2 Wh2} ----
            rj_sb = const.tile([128, 16], dt.float32)
            nc.scalar.activation(out=rj_sb[:], in_=wh2cols, func=A.Exp, scale=0.8)
            rjinv = const.tile([128, 16], dt.float32)
            nc.scalar.activation(out=rjinv[:], in_=wh2cols, func=A.Exp, scale=-0.8)
            ew2cols = const.tile([128, 16], dt.float32)
            nc.scalar.activation(out=ew2cols[:], in_=wh2cols, func=A.Exp, scale=0.2)

            # ---- rho_i broadcast [128 (bcast), i] in fp16 ----
            rib = const.tile([128, N], dt.float16)
            for c in range(4):
                pb = pp.tile([128, 512], dt.float32, tag="ps", name="ps")
                nc.tensor.matmul(out=pb[:], lhsT=ones_row[0:1, :],
                                 rhs=rows[0][0:1, c * 512:(c + 1) * 512],
                                 start=True, stop=True)
                nc.scalar.activation(out=rib[:, c * 512:(c + 1) * 512], in_=pb[:],
                                     func=A.Exp, scale=0.8)

            # ---- R tiles: R[j-tile][p, 0:128] = e^{0.2Wh2[j]} * Wh[j,:], col 128 = e^{0.2Wh2[j]} ----
            R_sb = const.tile([128, T, 132], dt.float16)
            for it in range(T):
                pR = pp.tile([128, 128], dt.float32, tag="ps", name="ps")
                nc.tensor.transpose(out=pR[:], in_=WhT[:, it * 128:(it + 1) * 128],
                                    identity=ident[:])
                nc.vector.tensor_scalar(out=R_sb[:, it, 0:128], in0=pR[:],
                                        scalar1=ew2cols[:, it:it + 1], scalar2=None,
                                        op0=Op.mult)
                nc.vector.tensor_copy(out=R_sb[:, it, 128:129],
                                      in_=ew2cols[:, it:it + 1])

        # ---- precompute mx_all[j-part, jt, i] = max(rho_i*rho_j, 1) in fp16 ----
        with tc.tile_pool(name="mxc", bufs=1) as mxc, \
             tc.tile_pool(name="panels", bufs=3) as panels, \
             tc.tile_pool(name="gp", bufs=2) as gp, \
             tc.tile_pool(name="stagp", bufs=2, space="PSUM") as stagp, \
             tc.tile_pool(name="accp", bufs=3, space="PSUM") as accp, \
             tc.tile_pool(name="lnp", bufs=8) as lnp, \
             tc.tile_pool(name="outp", bufs=3) as outp:

            mx_all = mxc.tile([128, T, N], dt.float16)
            for jt in range(T):
                nc.vector.tensor_scalar(out=mx_all[:, jt, :], in0=rib[:],
                                        scalar1=rjinv[:, jt:jt + 1],
                                        scalar2=rj_sb[:, jt:jt + 1],
                                        op0=Op.max, op1=Op.mult)

            # ---- main loop over adjacency ROW panels (i-major) ----
            adj_r = ADJ[:].rearrange("(t p) j -> p t j", p=128)

            for it in range(T):
                ap_ = panels.tile([128, N], dt.float16, tag="ap", name="ap")
                nc.gpsimd.dma_start(out=ap_[:], in_=adj_r[:, it, :])  # i32->f16 cast
                acc = accp.tile([128, 132], dt.float32, tag="acc", name="acc")
                g = gp.tile([128, T, 128], dt.float16, tag="g", name="g")
                for half in range(2):
                    stag = stagp.tile([128, 1024], dt.float16, tag="stag", name="stag")
                    for k in range(8):
                        jt = half * 8 + k
                        nc.tensor.transpose(out=stag[:, k * 128:(k + 1) * 128],
                                            in_=ap_[:, jt * 128:(jt + 1) * 128],
                                            identity=identh[:])
                    # g[j, jt, i-slice] = min(BIG*adjT, mx)
                    stag3 = stag[:].rearrange("p (k q) -> p k q", q=128)
                    nc.vector.scalar_tensor_tensor(
                        out=g[:, half * 8:(half + 1) * 8, :], in0=stag3,
                        scalar=BIG,
                        in1=mx_all[:, half * 8:(half + 1) * 8,
                                   it * 128:(it + 1) * 128],
                        op0=Op.mult, op1=Op.min)
                for jt in range(T):
                    nc.tensor.matmul(out=acc[:, 0:129], lhsT=g[:, jt, :],
                                     rhs=R_sb[:, jt, 0:129],
                                     start=(jt == 0), stop=(jt == T - 1))

                # ---- layernorm + final leaky + store ----
                y = acc[:, 0:128]
                Dc = acc[:, 128:129]
                stats = lnp.tile([128, 6], dt.float32, tag="stats", name="stats")
                nc.vector.bn_stats(out=stats[:], in_=y)
                mv = lnp.tile([128, 2], dt.float32, tag="mv", name="mv")
                nc.vector.bn_aggr(out=mv[:], in_=stats[:])
                dsb = lnp.tile([128, 1], dt.float32, tag="dsb", name="dsb")
                nc.vector.tensor_copy(out=dsb[:], in_=Dc)
                d2e = lnp.tile([128, 1], dt.float32, tag="d2e", name="d2e")
                nc.vector.scalar_tensor_tensor(out=d2e[:], in0=dsb[:], scalar=EPS,
                                               in1=dsb[:], op0=Op.mult, op1=Op.mult)
                lnv = lnp.tile([128, 1], dt.float32, tag="lnv", name="lnv")
                nc.scalar.activation(out=lnv[:], in_=mv[:, 1:2], func=A.Ln,
                                     bias=d2e[:], scale=1.0)
                rs = lnp.tile([128, 1], dt.float32, tag="rs", name="rs")
                nc.scalar.activation(out=rs[:], in_=lnv[:], func=A.Exp, scale=-0.5)
                nmrs = lnp.tile([128, 1], dt.float32, tag="nmrs", name="nmrs")
                nc.vector.tensor_scalar(out=nmrs[:], in0=mv[:, 0:1],
                                        scalar1=rs[:, 0:1], scalar2=-1.0,
                                        op0=Op.mult, op1=Op.mult)
                ot = outp.tile([128, 128], dt.float32, tag="ot", name="ot")
                nc.scalar.activation(out=ot[:], in_=y, func=A.Prelu,
                                     bias=nmrs[:], scale=rs[:, 0:1], alpha=ALPHA)
                nc.sync.dma_start(out=OUT[it * 128:(it + 1) * 128, :], in_=ot[:])

    if fix_waits:
        _fix_sync_waits(nc)
    return nc


_NC_CACHE = None


def _get_nc():
    global _NC_CACHE
    if _NC_CACHE is None:
        _NC_CACHE = build_gat_nc()
    return _NC_CACHE


def kernel(h, adj, W_w, W_b, a_w, _want_trace=False):
    h = np.ascontiguousarray(np.asarray(h, dtype=np.float32))
    adj = np.ascontiguousarray(np.asarray(adj, dtype=np.int32))
    W_w = np.ascontiguousarray(np.asarray(W_w, dtype=np.float32))
    W_b = np.ascontiguousarray(np.asarray(W_b, dtype=np.float32)).reshape(F, 1)
    a_w = np.ascontiguousarray(np.asarray(a_w, dtype=np.float32)).reshape(2, F)

    B = h.shape[0]
    assert B == N_CORES and h.shape == (B, N, F) and adj.shape == (B, N, N)

    nc = _get_nc()
    in_maps = [
        {"h": h[b], "adj": adj[b], "W_w": W_w, "W_b": W_b, "a_w": a_w}
        for b in range(B)
    ]
    res = run_bass_kernel_spmd(nc, in_maps, core_ids=list(range(N_CORES)),
                               trace=_want_trace)
    out = np.stack([res.results[b]["out"] for b in range(B)], axis=0)
    if _want_trace:
        return out, res
    return out
